# revision 23
# baseline (speedup 1.0000x reference)
"""DeepGraphSAGE (4x SAGEConv + BN/ReLU) on 8 Trainium2 NeuronCores.

Sharding: nodes partitioned across 8 cores (6250 dst nodes each).
Key structure vs the v1 kernel:
  - Each layer's AllGather is split into 4 quarter-chunks; edges are grouped
    by (dst block, src quarter) so gather+aggregate pipeline against the
    collective chunks as they arrive.
  - dma_gather uses prepare_only + trigger_dma on 4 SWDGE queues: descriptor
    generation (the serial gpsimd bottleneck) runs while collectives and
    earlier DMAs are in flight; the trigger carries the data dependency.
  - deginv is folded into the one-hot selection matrices (S values are
    deginv[dst] instead of 1.0), removing the normalize pass.
  - Aggregation accumulates across the 4 quarter groups in an SBUF f16
    buffer that is later overwritten by the pre-BN activations (aliased).
  - Layers 2/3/4 share one gather index/selection table (same edge order);
    gather indices are sorted ascending per chunk for HBM locality.
"""
import sys
import numpy as np

for p in ("/opt/trn_rl_repo",):
    if p not in sys.path:
        sys.path.append(p)

import concourse.bass as bass
import concourse.bacc as bacc
import concourse.mybir as mybir
from concourse.tile import TileContext
from concourse.masks import make_identity
from concourse.bass_utils import run_bass_kernel_spmd

f32 = mybir.dt.float32
f16 = mybir.dt.float16
i16 = mybir.dt.int16

NCORES = 8
P = 128
NQ = 4                 # SWDGE queues
USE_TRIGGER = False    # prepare_only + trigger_dma path
DEBUG_NO_CC = False    # replace AllGathers with local copies (hang bisect)
FUSE_DENSE = True      # emit dense per-tile inside the j=3 sweep
EPS = 1e-5
LAST_BUILD = None


# ---------------------------------------------------------------- host prep
class Grouping:
    """Edge grouping by (dst block, src quarter) for one table family.

    quarter_of(src) -> (j, table-relative row). Chunk counts are padded to
    the cross-core max so a single SPMD program fits every core; pad slots
    duplicate a valid index (S rows are zero there).
    """

    def __init__(self, n_own, nblk):
        self.n_own = n_own
        self.nblk = nblk
        self.kmax = None          # [(b,j)] -> padded chunk count
        self.calls = None         # list of (j, blocks, ktot, choff)

    @staticmethod
    def build(all_edges, n_own, nblk, quarter_fn, deginv_g):
        """all_edges: per-core list of (src_global, dst_local) arrays.
        quarter_fn(src)->(j, rel). Returns (grouping, per-core data dict)."""
        g = Grouping(n_own, nblk)
        ncores = len(all_edges)
        # per core, per (b, j): (rel_idx sorted, dst_in_block)
        percore = []
        for c in range(ncores):
            es, ed = all_edges[c]
            j_arr, rel = quarter_fn(es)
            blk = ed // P
            groups = {}
            for b in range(nblk):
                mb = blk == b
                for j in range(4):
                    m = mb & (j_arr == j)
                    r, dloc = rel[m], ed[m] - b * P
                    order = np.argsort(r, kind="stable")
                    groups[(b, j)] = (r[order], dloc[order])
            percore.append(groups)
        kmax = {}
        for b in range(nblk):
            for j in range(4):
                kmax[(b, j)] = max(
                    (len(percore[c][(b, j)][0]) + P - 1) // P for c in range(ncores)
                ) or 0
        g.kmax = kmax
        # calls: per (j, gtile of 4 blocks)
        calls = []
        choff = 0
        for j in range(4):
            for g0 in range(0, nblk, 4):
                blocks = list(range(g0, min(g0 + 4, nblk)))
                ktot = sum(kmax[(b, j)] for b in blocks)
                calls.append((j, blocks, ktot, choff))
                choff += ktot
        g.calls = calls
        g.totch = choff
        return g, percore

    def build_streams(self, percore_c, core, deginv_core):
        """idx stream [128, totch*8] i16 and S [128, totch, 128] f16 for one
        core. deginv_core: deginv for this core's own dst nodes [n_own]."""
        totch = self.totch
        iv = np.zeros(totch * P, np.int16)
        S = np.zeros((P, totch, P), np.float16)
        for (j, blocks, ktot, choff) in self.calls:
            off = choff
            for b in blocks:
                r, dloc = percore_c[(b, j)]
                k = self.kmax[(b, j)]
                n = len(r)
                for ci in range(k):
                    s0 = ci * P
                    v = r[s0:s0 + P]
                    base = off + ci
                    if len(v):
                        iv[base * P:base * P + len(v)] = v
                        if len(v) < P:  # pad with first idx of chunk
                            iv[base * P + len(v):(base + 1) * P] = v[0]
                        dl = dloc[s0:s0 + P]
                        S[np.arange(len(dl)), base, dl] = deginv_core[
                            b * P + dl].astype(np.float16)
                    else:  # all-pad chunk: gather row 0, S row zero
                        iv[base * P:(base + 1) * P] = 0
                off += k
        w = iv.reshape(-1, 16).T                    # [16, totch*8]
        idx16 = np.tile(w, (8, 1)).copy()           # [128, totch*8]
        return idx16, S


# ---------------------------------------------------------------- program
def build_program(n_nodes, in_f, hid, out_f, gA, gB, qlen):
    nown = n_nodes // NCORES
    nblk = (nown + P - 1) // P
    pad_n = nblk * P
    ntile = (nown + 511) // 512
    nfc = hid // P
    qoff = np.concatenate([[0], np.cumsum(qlen)])

    nc = bacc.Bacc("TRN2", target_bir_lowering=False, debug=False,
                   num_devices=NCORES, num_swdge_queues=NQ)

    # ---- I/O ----
    x16 = nc.dram_tensor("x16", [n_nodes, 128], f16, kind="ExternalInput")
    xT = nc.dram_tensor("xT", [in_f, pad_n], f16, kind="ExternalInput")
    idxA_d = nc.dram_tensor("idxA", [P, max(gA.totch * 8, 8)], i16, kind="ExternalInput")
    sA_d = nc.dram_tensor("sA", [P, max(gA.totch, 1), P], f16, kind="ExternalInput")
    idxB_d = nc.dram_tensor("idxB", [P, max(gB.totch * 8, 8)], i16, kind="ExternalInput")
    sB_d = nc.dram_tensor("sB", [P, max(gB.totch, 1), P], f16, kind="ExternalInput")
    wl_d, wr_d, g_d, b_d = {}, {}, {}, {}
    dims = [(in_f, hid), (hid, hid), (hid, hid), (hid, out_f)]
    for l, (fi, fo) in enumerate(dims, start=1):
        wl_d[l] = nc.dram_tensor(f"Wl{l}", [fi, fo], f16, kind="ExternalInput")
        wr_d[l] = nc.dram_tensor(f"Wr{l}", [fi, fo], f16, kind="ExternalInput")
    for l in (1, 2, 3):
        g_d[l] = nc.dram_tensor(f"g{l}", [hid], f32, kind="ExternalInput")
        b_d[l] = nc.dram_tensor(f"b{l}", [hid], f32, kind="ExternalInput")
    bl4_d = nc.dram_tensor("bl4", [out_f], f32, kind="ExternalInput")
    out_d = nc.dram_tensor("out", [nown, out_f], f32, kind="ExternalOutput")

    # ---- internal DRAM ----
    h_own = {j: nc.dram_tensor(f"hq{j}_own", [qlen[j], hid], f16) for j in range(4)}
    h_all = {j: nc.dram_tensor(f"hq{j}_all", [NCORES * qlen[j], hid], f16,
                               addr_space="Shared") for j in range(4)}
    y_own = {j: nc.dram_tensor(f"yq{j}_own", [qlen[j], 128], f16) for j in range(4)}
    y_all = {j: nc.dram_tensor(f"yq{j}_all", [NCORES * qlen[j], 128], f16,
                               addr_space="Shared") for j in range(4)}
    st_in = {l: nc.dram_tensor(f"st{l}_in", [P, 8], f32) for l in (1, 2, 3)}
    st_out = {l: nc.dram_tensor(f"st{l}_out", [P, 8], f32, addr_space="Shared")
              for l in (1, 2, 3)}
    rg = [list(range(NCORES))]

    dma_sems = None

    with TileContext(nc) as tc:
        with (
            tc.tile_pool(name="const", bufs=1) as cp,
            tc.tile_pool(name="gbuf", bufs=3) as gp,
            tc.tile_pool(name="sbuf", bufs=3) as sp,
            tc.tile_pool(name="small", bufs=2) as sm,
            tc.tile_pool(name="aggt", bufs=2) as at,
            tc.tile_pool(name="psA", bufs=3, space="PSUM") as psA,
            tc.tile_pool(name="psB", bufs=2, space="PSUM") as psB,
            tc.tile_pool(name="psC", bufs=2, space="PSUM") as psC,
        ):
            dma_sems = [nc.alloc_semaphore(f"swdge_dma{q}", num=240 + q) for q in range(NQ)]
            ident = cp.tile([P, P], f16)
            make_identity(nc, ident[:])
            ident32 = cp.tile([P, P], f32)
            make_identity(nc, ident32[:])
            # weights resident in SBUF
            W = {}
            for l, (fi, fo) in enumerate(dims, start=1):
                kc = (fi + P - 1) // P
                for (nm, dram) in (("l", wl_d[l]), ("r", wr_d[l])):
                    for q in range(kc):
                        r0, r1 = q * P, min((q + 1) * P, fi)
                        t = cp.tile([r1 - r0, fo], f16, tag=f"W{nm}{l}_{q}")
                        nc.sync.dma_start(out=t[:], in_=dram[r0:r1, :])
                        W[(nm, l, q)] = t
            gb = {}
            for l in (1, 2, 3):
                for nm, dram in (("g", g_d[l]), ("b", b_d[l])):
                    t = cp.tile([P, nfc], f32, tag=f"{nm}{l}")
                    nc.sync.dma_start(out=t[:], in_=dram[:].rearrange("(c p) -> p c", p=P))
                    gb[(nm, l)] = t
            bl4_t = cp.tile([P, 1], f32)
            nc.sync.dma_start(out=bl4_t[:out_f, :], in_=bl4_d[:, None])

            # acc/preBN buffer (aliased) and hT buffer, tile-major layout:
            # bufA col b*512 == (tile t=b//4) region; preBN(q,t) at t*2048+q*512
            bufA = cp.tile([P, ntile * nfc * 512], f16, name="bufA")
            bufB = cp.tile([P, ntile * nfc * 512], f16, name="bufB")

            qcnt = [0]  # round-robin over gather queues
            dma_counts = [0] * NQ  # cumulative dma-sem targets per queue

            def acc_slice(b, width):
                return bufA[:, b * 512:b * 512 + width]

            def preBN_slice(q, t, n0, n1):
                base = t * 2048 + q * 512
                return bufA[:, base + n0:base + n1]

            def hT_slice(q, t, n0, n1):
                base = t * 2048 + q * 512
                return bufB[:, base + n0:base + n1]

            def aggregate(grouping, idx_tile, idx_dram, s_dram, tables, width,
                          row_elems, use_trigger, on_tile=None):
                """Gather+aggregate all 4 quarter groups into bufA (f16).

                Triggers lag the preps by AHEAD calls so descriptor generation
                overlaps the AG wait / DMA drain. Data-ready is enforced by an
                explicit wait_ge on the consumer (PE) — Tile does not insert
                it for prepare-mode gathers. on_tile(t) fires after tile t's
                accumulation completes (j==3 consumed).
                """
                first_seen = set()
                AHEAD = 2  # must stay < NQ (trigger count=None fires per queue)

                def consume(j, blocks, ktot, choff, S, G, q, target):
                    if target is not None:
                        nc.tensor.wait_ge(dma_sems[q], target)
                    off = choff
                    for b in blocks:
                        k = grouping.kmax[(b, j)]
                        if k == 0:
                            continue
                        ps = psA.tile([P, 512], f32, tag="aggps")
                        for ci in range(k):
                            cj = off - choff + ci
                            nc.tensor.matmul(
                                out=ps[:, :width],
                                lhsT=S[:, cj, :], rhs=G[:, cj, :width],
                                start=(ci == 0), stop=(ci == k - 1),
                            )
                        dstap = acc_slice(b, width)
                        if b not in first_seen:
                            first_seen.add(b)
                            nc.vector.tensor_copy(out=dstap[:], in_=ps[:, :width])
                        else:
                            nc.vector.tensor_tensor(
                                out=dstap[:], in0=dstap[:],
                                in1=ps[:, :width], op=mybir.AluOpType.add)
                        off += k
                    if j == 3 and on_tile is not None:
                        on_tile(blocks[0] // 4)

                pending = []

                def flush():
                    args = pending.pop(0)
                    q = args[6]
                    nc.gpsimd.trigger_dma(count=None, queue_num=q)
                    dma_counts[q] += 16
                    consume(*args, dma_counts[q])

                for (j, blocks, ktot, choff) in grouping.calls:
                    if ktot == 0:
                        continue
                    q = qcnt[0] % NQ
                    qcnt[0] += 1
                    G = gp.tile([P, ktot, row_elems], f16, tag=f"G{row_elems}")
                    S = sp.tile([P, ktot, P], f16, tag="S")
                    nc.scalar.dma_start(out=S[:], in_=s_dram[:, choff:choff + ktot, :])
                    if idx_tile is not None:
                        idx_ap = idx_tile[:, choff * 8:(choff + ktot) * 8]
                    else:
                        it = sm.tile([P, ktot * 8], i16, tag="idxs")
                        nc.scalar.dma_start(out=it[:],
                                            in_=idx_dram[:, choff * 8:(choff + ktot) * 8])
                        idx_ap = it[:]
                    if use_trigger:
                        nc.gpsimd.dma_gather(
                            out_ap=G[:], in_ap=tables[j],
                            idxs_ap=idx_ap,
                            num_idxs=ktot * P, num_idxs_reg=ktot * P,
                            elem_size=row_elems, single_packet=False,
                            prepare_only=True, sem=dma_sems[q], queue_num=q,
                        )
                        if len(pending) >= AHEAD:
                            flush()
                        pending.append((j, blocks, ktot, choff, S, G, q))
                    else:
                        nc.gpsimd.dma_gather(
                            out_ap=G[:], in_ap=tables[j],
                            idxs_ap=idx_ap,
                            num_idxs=ktot * P, num_idxs_reg=ktot * P,
                            elem_size=row_elems, single_packet=False,
                            queue_num=q,
                        )
                        consume(j, blocks, ktot, choff, S, G, q, None)
                while pending:
                    flush()

            def make_aggT(t, fi_chunks, width):
                """Transpose acc blocks of tile t into aggT tiles [fi, 512]."""
                blocks = range(4 * t, min(4 * t + 4, nblk))
                aggT = [at.tile([P, 512], f16, tag=f"aggT{q}", name=f"aggT{q}_{t}")
                        for q in range(fi_chunks)]
                for bi, b in enumerate(blocks):
                    tp = psB.tile([P, 512], f16, tag="tp")
                    for q in range(fi_chunks):
                        w0 = q * P
                        w1 = min(w0 + P, width)
                        if w0 >= width:
                            break
                        nc.tensor.matmul(out=tp[:w1 - w0, q * P:q * P + P],
                                         lhsT=acc_slice(b, 512)[:, w0:w1],
                                         rhs=ident[:], is_transpose=True)
                        nc.vector.tensor_copy(
                            out=aggT[q][:w1 - w0, bi * P:(bi + 1) * P],
                            in_=tp[:w1 - w0, q * P:q * P + P])
                return aggT

            def make_dense_cb(l, fi_chunks, width, rhs_root):
                """Per-tile dense (agg@Wl + root@Wr) -> stats -> preBN."""
                stats = [sm.tile([P, ntile * 6], f32, tag=f"stats{q}",
                                 name=f"stats{l}_{q}") for q in range(nfc)]

                def dense_tile(t):
                    ns, ne = t * 512, min((t + 1) * 512, nown)
                    nn = ne - ns
                    aggT = make_aggT(t, fi_chunks, width)
                    roots = rhs_root(t, ns, ne)
                    for fo in range(nfc):
                        dps = psC.tile([P, 512], f32, tag="dense")
                        nmm = 2 * fi_chunks
                        mm = 0
                        for q in range(fi_chunks):
                            w0 = q * P
                            w1 = min(w0 + P, width)
                            nc.tensor.matmul(out=dps[:, :nn],
                                             lhsT=W[("l", l, q)][:, fo * P:(fo + 1) * P],
                                             rhs=aggT[q][:w1 - w0, :nn],
                                             start=(mm == 0), stop=(mm == nmm - 1))
                            mm += 1
                            nc.tensor.matmul(out=dps[:, :nn],
                                             lhsT=W[("r", l, q)][:, fo * P:(fo + 1) * P],
                                             rhs=roots[q],
                                             start=False, stop=(mm == nmm - 1))
                            mm += 1
                        nc.vector.bn_stats(out=stats[fo][:, t * 6:(t + 1) * 6],
                                           in_=dps[:, :nn])
                        nc.vector.tensor_copy(out=preBN_slice(fo, t, 0, nn),
                                              in_=dps[:, :nn])
                return stats, dense_tile

            def bn_allreduce(l, stats):
                pack = sm.tile([P, 8], f32, tag="pack")
                for q in range(nfc):
                    mv = sm.tile([P, 2], f32, tag="mv")
                    nc.vector.bn_aggr(out=mv[:], in_=stats[q][:])
                    sq = sm.tile([P, 1], f32, tag="sq")
                    nc.vector.tensor_tensor(out=sq[:], in0=mv[:, 0:1],
                                            in1=mv[:, 0:1], op=mybir.AluOpType.mult)
                    nc.vector.tensor_tensor(out=sq[:], in0=sq[:], in1=mv[:, 1:2],
                                            op=mybir.AluOpType.add)
                    nc.vector.tensor_scalar(out=pack[:, 2 * q:2 * q + 1],
                                            in0=mv[:, 0:1], scalar1=float(nown),
                                            scalar2=None, op0=mybir.AluOpType.mult)
                    nc.vector.tensor_scalar(out=pack[:, 2 * q + 1:2 * q + 2],
                                            in0=sq[:], scalar1=float(nown),
                                            scalar2=None, op0=mybir.AluOpType.mult)
                nc.sync.dma_start(out=st_in[l][:, :], in_=pack[:])
                if DEBUG_NO_CC:
                    nc.sync.dma_start(out=st_out[l][:, :], in_=st_in[l][:, :])
                else:
                    nc.gpsimd.collective_compute(
                        "AllReduce", mybir.AluOpType.add, replica_groups=rg,
                        ins=[st_in[l][:, :]], outs=[st_out[l][:, :]],
                    )
                red = sm.tile([P, 8], f32, tag="red")
                nc.sync.dma_start(out=red[:], in_=st_out[l][:, :])
                scale = sm.tile([P, nfc], f32, tag="scale", name=f"scale{l}")
                shift = sm.tile([P, nfc], f32, tag="shift", name=f"shift{l}")
                inv_n = 1.0 / float(n_nodes)
                for q in range(nfc):
                    mu = sm.tile([P, 1], f32, tag="mu")
                    var = sm.tile([P, 1], f32, tag="var")
                    nc.vector.tensor_scalar(out=mu[:], in0=red[:, 2 * q:2 * q + 1],
                                            scalar1=inv_n, scalar2=None,
                                            op0=mybir.AluOpType.mult)
                    nc.vector.tensor_scalar(out=var[:], in0=red[:, 2 * q + 1:2 * q + 2],
                                            scalar1=inv_n, scalar2=None,
                                            op0=mybir.AluOpType.mult)
                    musq = sm.tile([P, 1], f32, tag="musq")
                    nc.vector.tensor_tensor(out=musq[:], in0=mu[:], in1=mu[:],
                                            op=mybir.AluOpType.mult)
                    nc.vector.tensor_tensor(out=var[:], in0=var[:], in1=musq[:],
                                            op=mybir.AluOpType.subtract)
                    nc.vector.tensor_scalar(out=var[:], in0=var[:], scalar1=EPS,
                                            scalar2=None, op0=mybir.AluOpType.add)
                    nc.vector.reciprocal(out=var[:], in_=var[:])
                    rs = sm.tile([P, 1], f32, tag="rs")
                    nc.scalar.activation(out=rs[:], in_=var[:],
                                         func=mybir.ActivationFunctionType.Sqrt)
                    nc.vector.tensor_tensor(out=scale[:, q:q + 1], in0=rs[:],
                                            in1=gb[("g", l)][:, q:q + 1],
                                            op=mybir.AluOpType.mult)
                    nc.vector.tensor_tensor(out=musq[:], in0=mu[:],
                                            in1=scale[:, q:q + 1],
                                            op=mybir.AluOpType.mult)
                    nc.vector.tensor_tensor(out=shift[:, q:q + 1],
                                            in0=gb[("b", l)][:, q:q + 1], in1=musq[:],
                                            op=mybir.AluOpType.subtract)
                return scale, shift

            def bn_rows_ag(l, scale, shift):
                """BN+ReLU preBN->hT; transpose to rows; DMA; quarter AGs."""
                for t in range(ntile):
                    ns, ne = t * 512, min((t + 1) * 512, nown)
                    nn = ne - ns
                    for q in range(nfc):
                        nc.scalar.activation(
                            out=hT_slice(q, t, 0, nn), in_=preBN_slice(q, t, 0, nn),
                            func=mybir.ActivationFunctionType.Relu,
                            bias=shift[:, q:q + 1], scale=scale[:, q:q + 1],
                        )
                # rows per block, grouped by quarter; AG fires per quarter
                b0 = 0
                for j in range(4):
                    rows_in_q = qlen[j]
                    nb = (rows_in_q + P - 1) // P
                    for bi in range(nb):
                        b = b0 + bi
                        t, i = b // 4, b % 4
                        r0 = bi * P
                        r1 = min(r0 + P, rows_in_q)
                        tpr = psB.tile([P, 512], f16, tag="tp")
                        for q in range(nfc):
                            nc.tensor.matmul(
                                out=tpr[:, q * P:(q + 1) * P],
                                lhsT=hT_slice(q, t, i * P, i * P + P)[:, :],
                                rhs=ident[:], is_transpose=True)
                        rows = sm.tile([P, hid], f16, tag="rows")
                        nc.vector.tensor_copy(out=rows[:], in_=tpr[:, :hid])
                        nc.sync.dma_start(out=h_own[j][r0:r1, :],
                                          in_=rows[:r1 - r0, :])
                    b0 += nb
                    if DEBUG_NO_CC:
                        nc.sync.dma_start(out=h_all[j][0:qlen[j], :],
                                          in_=h_own[j][:, :])
                    else:
                        nc.gpsimd.collective_compute(
                            "AllGather", mybir.AluOpType.bypass, replica_groups=rg,
                            ins=[h_own[j][:, :]], outs=[h_all[j][:, :]],
                        )

            # ================= layer 1 =================
            x_tables = [x16[12500 * j:12500 * (j + 1), :] for j in range(4)]

            def xT_root(t, ns, ne):
                xt = sm.tile([in_f, 512], f16, tag="xTt")
                nc.sync.dma_start(out=xt[:, :ne - ns], in_=xT[:, ns:ne])
                return [xt[:, :ne - ns]]

            stats, dense_cb = make_dense_cb(1, 1, in_f, xT_root)
            aggregate(gA, None, idxA_d, sA_d, x_tables, in_f, 128,
                      use_trigger=USE_TRIGGER, on_tile=dense_cb if FUSE_DENSE else None)
            if not FUSE_DENSE:
                for _t in range(ntile):
                    dense_cb(_t)
            scale, shift = bn_allreduce(1, stats)
            bn_rows_ag(1, scale, shift)

            # ================= layers 2,3 =================
            for l in (2, 3):
                h_tables = [h_all[j][:, :] for j in range(4)]

                def h_root(t, ns, ne, _l=l):
                    return [hT_slice(q, t, 0, ne - ns) for q in range(nfc)]

                stats, dense_cb = make_dense_cb(l, nfc, hid, h_root)
                aggregate(gB, None, idxB_d, sB_d, h_tables, hid, hid,
                          use_trigger=USE_TRIGGER, on_tile=dense_cb if FUSE_DENSE else None)
                if not FUSE_DENSE:
                    for _t in range(ntile):
                        dense_cb(_t)
                scale, shift = bn_allreduce(l, stats)
                bn_rows_ag(l, scale, shift)

            # ================= layer 4 =================
            # y = h3 @ Wl4 (transposed), to rows, quarter AGs
            for t in range(ntile):
                ns, ne = t * 512, min((t + 1) * 512, nown)
                nn = ne - ns
                yps = psC.tile([P, 512], f32, tag="dense")
                for q in range(nfc):
                    nc.tensor.matmul(out=yps[:out_f, :nn],
                                     lhsT=W[("l", 4, q)][:, :out_f],
                                     rhs=hT_slice(q, t, 0, nn),
                                     start=(q == 0), stop=(q == nfc - 1))
                ysb = sm.tile([P, 512], f16, tag="ysb")
                nc.vector.tensor_copy(out=ysb[:out_f, :nn], in_=yps[:out_f, :nn])
                for bi in range((nn + P - 1) // P):
                    c0 = bi * P
                    c1 = min(c0 + P, nn)
                    tpy = psB.tile([P, 512], f16, tag="tp")
                    nc.tensor.matmul(out=tpy[:c1 - c0, :out_f],
                                     lhsT=ysb[:out_f, c0:c1],
                                     rhs=ident[:out_f, :out_f],
                                     is_transpose=True)
                    yr = sm.tile([P, 128], f16, tag="yrows")
                    nc.vector.memset(yr[:], 0.0)
                    nc.vector.tensor_copy(out=yr[:c1 - c0, :out_f],
                                          in_=tpy[:c1 - c0, :out_f])
                    b = 4 * t + bi
                    # global row b*128+... falls in quarter j at offset r0
                    gr0 = b * P
                    j = int(np.searchsorted(qoff, gr0, side="right") - 1)
                    r0 = gr0 - qoff[j]
                    nc.sync.dma_start(out=y_own[j][r0:r0 + (c1 - c0), :],
                                      in_=yr[:c1 - c0, :])
            for j in range(4):
                if DEBUG_NO_CC:
                    nc.sync.dma_start(out=y_all[j][0:qlen[j], :],
                                      in_=y_own[j][:, :])
                else:
                    nc.gpsimd.collective_compute(
                        "AllGather", mybir.AluOpType.bypass, replica_groups=rg,
                        ins=[y_own[j][:, :]], outs=[y_all[j][:, :]],
                    )
            # aggregate y (quartered), reusing the B grouping/streams, and
            # emit the output tail per tile as its accumulation completes
            y_tables = [y_all[j][:, :] for j in range(4)]

            def l4_tail(t):
                ns, ne = t * 512, min((t + 1) * 512, nown)
                nn = ne - ns
                aggT = make_aggT(t, 1, out_f)
                ops = psC.tile([P, 512], f32, tag="dense")
                for q in range(nfc):
                    nc.tensor.matmul(out=ops[:out_f, :nn],
                                     lhsT=W[("r", 4, q)][:, :out_f],
                                     rhs=hT_slice(q, t, 0, nn),
                                     start=(q == 0), stop=(q == nfc - 1))
                osb = sm.tile([P, 512], f32, tag="osb")
                nc.vector.tensor_tensor(out=osb[:out_f, :nn], in0=ops[:out_f, :nn],
                                        in1=aggT[0][:out_f, :nn],
                                        op=mybir.AluOpType.add)
                nc.vector.tensor_scalar(out=osb[:out_f, :nn], in0=osb[:out_f, :nn],
                                        scalar1=bl4_t[:out_f, 0:1], scalar2=None,
                                        op0=mybir.AluOpType.add)
                for bi in range((nn + P - 1) // P):
                    c0, c1 = bi * P, min(bi * P + P, nn)
                    tpo = psB.tile([P, 512], f32, tag="tp")
                    nc.tensor.matmul(out=tpo[:c1 - c0, :out_f],
                                     lhsT=osb[:out_f, c0:c1],
                                     rhs=ident32[:out_f, :out_f],
                                     is_transpose=True)
                    orow = sm.tile([P, out_f], f32, tag="orow")
                    nc.vector.tensor_copy(out=orow[:c1 - c0, :],
                                          in_=tpo[:c1 - c0, :out_f])
                    nc.sync.dma_start(out=out_d[ns + c0:ns + c1, :],
                                      in_=orow[:c1 - c0, :])

            aggregate(gB, None, idxB_d, sB_d, y_tables, out_f, 128,
                      use_trigger=USE_TRIGGER, on_tile=l4_tail if FUSE_DENSE else None)
            if not FUSE_DENSE:
                for _t in range(ntile):
                    l4_tail(_t)
    return nc


def kernel(**inputs):
    x = np.asarray(inputs["x"], np.float32)
    edge_index = np.asarray(inputs["edge_index"])
    n_nodes, in_f = x.shape
    hid = inputs["Wl2"].shape[0]
    out_f = inputs["Wl4"].shape[1]
    nown = n_nodes // NCORES
    nblk = (nown + P - 1) // P

    src = np.asarray(edge_index[0]).astype(np.int64)
    dst = np.asarray(edge_index[1]).astype(np.int64)
    deg = np.bincount(dst, minlength=n_nodes).astype(np.float32)
    deginv = (1.0 / np.maximum(deg, 1.0)).astype(np.float32)

    # quarter lengths of the per-core row space (block-aligned except last)
    q_blocks = [13, 13, 13, nblk - 39]
    qlen = []
    off = 0
    for nbq in q_blocks:
        ln = min(nbq * P, nown - off)
        qlen.append(ln)
        off += ln
    qoff = np.concatenate([[0], np.cumsum(qlen)])  # [0,1664,3328,4992,6250]

    # per-core edge lists (dst-sharded)
    all_edges = []
    for c in range(NCORES):
        lo = c * nown
        m = (dst >= lo) & (dst < lo + nown)
        all_edges.append((src[m], dst[m] - lo))

    # grouping A: src quartered by global node range (for x table)
    def quarter_global(s):
        j = np.minimum(s // 12500, 3).astype(np.int64)
        return j, (s - j * 12500).astype(np.int64)

    # grouping B: src quartered by (owner core, local quarter)
    def quarter_local(s):
        c = s // nown
        r = s - c * nown
        j = np.searchsorted(qoff, r, side="right") - 1
        rel = c * np.array(qlen)[j] + (r - qoff[j])
        return j.astype(np.int64), rel.astype(np.int64)

    gA, pcA = Grouping.build(all_edges, nown, nblk, quarter_global, deginv)
    gB, pcB = Grouping.build(all_edges, nown, nblk, quarter_local, deginv)

    import time as _time
    _t0 = _time.perf_counter()
    nc = build_program(n_nodes, in_f, hid, out_f, gA, gB, qlen)
    print(f"[kernel] program built in {_time.perf_counter() - _t0:.1f}s", flush=True)
    _t0 = _time.perf_counter()
    nc.compile()
    print(f"[kernel] bacc compile in {_time.perf_counter() - _t0:.1f}s", flush=True)

    x16 = np.zeros((n_nodes, 128), np.float16)
    x16[:, :in_f] = x.astype(np.float16)
    pad_n = nblk * P

    in_maps = []
    for c in range(NCORES):
        dgc = deginv[c * nown:(c + 1) * nown]
        idxA, sA = gA.build_streams(pcA[c], c, dgc)
        idxB, sB = gB.build_streams(pcB[c], c, dgc)
        xTc = np.zeros((in_f, pad_n), np.float16)
        xTc[:, :nown] = x[c * nown:(c + 1) * nown].T.astype(np.float16)
        im = {
            "x16": x16, "xT": xTc,
            "idxA": idxA if idxA.size else np.zeros((P, 8), np.int16),
            "sA": sA if sA.size else np.zeros((P, 1, P), np.float16),
            "idxB": idxB if idxB.size else np.zeros((P, 8), np.int16),
            "sB": sB if sB.size else np.zeros((P, 1, P), np.float16),
            "bl4": np.asarray(inputs["bl4"], np.float32),
        }
        for l in (1, 2, 3, 4):
            im[f"Wl{l}"] = np.asarray(inputs[f"Wl{l}"], np.float16)
            im[f"Wr{l}"] = np.asarray(inputs[f"Wr{l}"], np.float16)
        for l in (1, 2, 3):
            im[f"g{l}"] = np.asarray(inputs[f"g{l}"], np.float32)
            im[f"b{l}"] = np.asarray(inputs[f"b{l}"], np.float32)
        in_maps.append(im)

    global LAST_BUILD
    LAST_BUILD = (nc, in_maps)
    res = run_bass_kernel_spmd(nc, in_maps, list(range(NCORES)))
    out = np.concatenate([res.results[c]["out"] for c in range(NCORES)], axis=0)
    return out.astype(np.float32)


# revision 28
# speedup vs baseline: 1.2598x; 1.2598x over previous
"""DeepGraphSAGE (4x SAGEConv + BN/ReLU) on 8 Trainium2 NeuronCores.

Sharding: nodes partitioned across 8 cores (6250 dst nodes each). Each layer:
  - mean-aggregate neighbor features via dma_gather (rows of the allgathered
    H table) + one-hot selection matmuls accumulating in PSUM
  - dense transforms computed in transposed layout (features on partitions)
  - BatchNorm stats via bn_stats/bn_aggr + tiny cross-core AllReduce
  - PE transposes back to row layout, AllGather of H for the next layer.
Data is fp16 on the wire and in matmuls; accumulation/stats are fp32.
"""
import sys
import numpy as np

for p in ("/opt/trn_rl_repo",):
    if p not in sys.path:
        sys.path.append(p)

import concourse.bass as bass
import concourse.bacc as bacc
import concourse.mybir as mybir
from concourse.tile import TileContext
from concourse.masks import make_identity
from concourse.bass_utils import run_bass_kernel_spmd

f32 = mybir.dt.float32
f16 = mybir.dt.float16
i16 = mybir.dt.int16

NCORES = 8
P = 128
SPLIT = 32768          # int16 index limit
BASE2 = 17232          # second gather base (recomputed per problem size)
EPS = 1e-5
LAST_BUILD = None


# ---------------------------------------------------------------- host prep
class Plan:
    """Per-core gather/selection plan derived from edge_index."""

    def __init__(self, n_nodes, src, dst, core):
        self.n_own = n_nodes // NCORES
        self.nblk = (self.n_own + P - 1) // P
        lo = core * self.n_own
        m = (dst >= lo) & (dst < lo + self.n_own)
        es = src[m].astype(np.int64)
        ed = (dst[m] - lo).astype(np.int64)
        order = np.argsort(ed, kind="stable")
        es, ed = es[order], ed[order]
        bounds = np.searchsorted(ed, np.arange(0, self.nblk + 1) * P)

        idx_vals = []     # flat int16 index stream (multiple of 128 per group)
        dj_vals = []      # flat f16 dst-local stream, same order (200=pad)
        calls = []        # per PAIR: [(base_id, [k per block in pair]), ...]
        npair = (self.nblk + 1) // 2
        for pr in range(npair):
            blocks = [b for b in (2 * pr, 2 * pr + 1) if b < self.nblk]
            groups = []
            for base_id in (0, 1):
                ks = []
                for b in blocks:
                    e0, e1 = bounds[b], bounds[b + 1]
                    bs, bd = es[e0:e1], ed[e0:e1] - b * P
                    msel = (bs < SPLIT) if base_id == 0 else (bs >= SPLIT)
                    gs, gd = bs[msel], bd[msel]
                    k = (len(gs) + P - 1) // P
                    ks.append(k)
                    if k == 0:
                        continue
                    padded = np.zeros(k * P, np.int64)
                    padded[: len(gs)] = gs - (BASE2 if base_id else 0)
                    djp = np.full(k * P, 200.0, np.float32)
                    djp[: len(gs)] = gd.astype(np.float32)
                    # sort each chunk's slots by source for HBM locality
                    for j in range(k):
                        sl = slice(j * P, min((j + 1) * P, len(gs)))
                        n = sl.stop - sl.start
                        if n > 1:
                            o = np.argsort(padded[sl], kind="stable")
                            padded[sl] = padded[sl][o]
                            djp[j * P:j * P + n] = djp[j * P:j * P + n][o]
                    idx_vals.append(padded.astype(np.int16))
                    dj_vals.append(djp)
                groups.append((base_id, ks))
            calls.append(groups)

        self.calls = calls
        self.npair = npair
        totch = sum(len(d) for d in dj_vals) // P
        self.totch = totch
        iv = np.concatenate(idx_vals) if idx_vals else np.zeros(0, np.int16)
        # dma_gather index layout: position i -> [i%16, i//16], replicated 8x
        w = iv.reshape(-1, 16).T  # [16, totch*8]
        self.idx16 = np.tile(w, (8, 1)).copy()           # [128, totch*8] i16
        djf = np.concatenate(dj_vals) if dj_vals else np.zeros(0, np.float32)
        self.dj16 = djf.reshape(-1, P).T.copy()          # [128, totch] f32


def _plan_all(n_nodes, edge_index):
    global BASE2
    BASE2 = max(0, n_nodes - SPLIT)
    src = np.asarray(edge_index[0])
    dst = np.asarray(edge_index[1])
    return [Plan(n_nodes, src, dst, c) for c in range(NCORES)]


# ---------------------------------------------------------------- program
def build_program(n_nodes, in_f, hid, out_f, plan0):
    """One SPMD program (same for all cores; per-core data differs)."""
    nown = plan0.n_own
    nblk = plan0.nblk
    pad_n = nblk * P
    ntile = (nown + 511) // 512
    nhalf = nown // 2
    nhalftot = NCORES * nhalf
    nfc = hid // P               # 4 feature chunks of the hidden dim
    totch = plan0.totch
    calls = plan0.calls

    nc = bacc.Bacc("TRN2", target_bir_lowering=False, debug=False,
                   num_devices=NCORES, num_swdge_queues=2)

    # ---- I/O ----
    x16 = nc.dram_tensor("x16", [n_nodes, 128], f16, kind="ExternalInput")
    xT = nc.dram_tensor("xT", [in_f, pad_n], f16, kind="ExternalInput")
    idx16_d = nc.dram_tensor("idx16", [P, max(totch * 8, 8)], i16, kind="ExternalInput")
    dj_d = nc.dram_tensor("dj", [P, max(totch, 1)], f32, kind="ExternalInput")
    deginv_d = nc.dram_tensor("deginv", [pad_n], f32, kind="ExternalInput")
    wl_d, wr_d, g_d, b_d = {}, {}, {}, {}
    dims = [(in_f, hid), (hid, hid), (hid, hid), (hid, out_f)]
    for l, (fi, fo) in enumerate(dims, start=1):
        wl_d[l] = nc.dram_tensor(f"Wl{l}", [fi, fo], f16, kind="ExternalInput")
        wr_d[l] = nc.dram_tensor(f"Wr{l}", [fi, fo], f16, kind="ExternalInput")
    for l in (1, 2, 3):
        g_d[l] = nc.dram_tensor(f"g{l}", [hid], f32, kind="ExternalInput")
        b_d[l] = nc.dram_tensor(f"b{l}", [hid], f32, kind="ExternalInput")
    bl4_d = nc.dram_tensor("bl4", [out_f], f32, kind="ExternalInput")
    out_d = nc.dram_tensor("out", [nown, out_f], f32, kind="ExternalOutput")

    # ---- internal DRAM ----
    h_own = {l: nc.dram_tensor(f"h{l}_own", [nown, hid], f16) for l in (1, 2, 3)}
    h_all = {l: nc.dram_tensor(f"h{l}_all", [n_nodes, hid], f16, addr_space="Shared")
             for l in (1, 2, 3)}
    y_own = nc.dram_tensor("y_own", [nown, 128], f16)
    y_all = nc.dram_tensor("y_all", [n_nodes, 128], f16, addr_space="Shared")
    st_in = {l: nc.dram_tensor(f"st{l}_in", [P, 8], f32) for l in (1, 2, 3)}
    st_out = {l: nc.dram_tensor(f"st{l}_out", [P, 8], f32, addr_space="Shared")
              for l in (1, 2, 3)}
    rg = [list(range(NCORES))]

    with TileContext(nc) as tc:
        with (
            tc.tile_pool(name="const", bufs=1) as cp,
            tc.tile_pool(name="sbuf", bufs=2) as sb,
            tc.tile_pool(name="small", bufs=3) as sm,
            tc.tile_pool(name="psA", bufs=2, space="PSUM") as psA,
            tc.tile_pool(name="psB", bufs=2, space="PSUM") as psB,
            tc.tile_pool(name="psC", bufs=2, space="PSUM") as psC,
        ):
            ident = cp.tile([P, P], f16)
            make_identity(nc, ident[:])
            ident32 = cp.tile([P, P], f32)
            make_identity(nc, ident32[:])
            iota_i = cp.tile([P, P], mybir.dt.int32)
            nc.gpsimd.iota(iota_i[:], pattern=[[1, P]], base=0,
                           channel_multiplier=0)
            iota_t = cp.tile([P, P], f16)
            nc.vector.tensor_copy(out=iota_t[:], in_=iota_i[:])
            deginv_t = cp.tile([P, nblk], f32)
            nc.sync.dma_start(out=deginv_t[:],
                              in_=deginv_d[:].rearrange("(b p) -> p b", p=P))
            # weights resident in SBUF, per fi-chunk tiles
            W = {}
            for l, (fi, fo) in enumerate(dims, start=1):
                kc = (fi + P - 1) // P
                for (nm, dram) in (("l", wl_d[l]), ("r", wr_d[l])):
                    for q in range(kc):
                        r0, r1 = q * P, min((q + 1) * P, fi)
                        t = cp.tile([r1 - r0, fo], f16, tag=f"W{nm}{l}_{q}")
                        nc.sync.dma_start(out=t[:], in_=dram[r0:r1, :])
                        W[(nm, l, q)] = t
            gb = {}
            for l in (1, 2, 3):
                for nm, dram in (("g", g_d[l]), ("b", b_d[l])):
                    t = cp.tile([P, nfc], f32, tag=f"{nm}{l}")
                    nc.sync.dma_start(out=t[:], in_=dram[:].rearrange("(c p) -> p c", p=P))
                    gb[(nm, l)] = t
            bl4_t = cp.tile([P, 1], f32)
            nc.sync.dma_start(out=bl4_t[:out_f, :], in_=bl4_d[:, None])

            # persistent hidden state (transposed) + pre-BN buffer
            hT = [cp.tile([P, pad_n], f16, tag=f"hT{q}", name=f"hT{q}") for q in range(nfc)]
            preBN = [cp.tile([P, pad_n], f16, tag=f"preBN{q}", name=f"preBN{q}") for q in range(nfc)]

            gq = [0]  # gather queue round-robin state

            def aggregate_pair(pr, src_table, src_table2, width, tagsfx, row_elems):
                """Mean-aggregate both blocks of pair pr. One dma_gather per
                base-group spanning the pair. Returns list of f16 tiles."""
                groups = calls[pr]
                blocks = [b for b in (2 * pr, 2 * pr + 1) if b < nblk]
                ktot = sum(sum(ks) for _, ks in groups)
                out_tiles = []
                if ktot == 0:
                    for bi in range(len(blocks)):
                        z = sm.tile([P, width], f16, tag=f"agg{tagsfx}{bi}",
                                    name=f"aggz{bi}")
                        nc.vector.memset(z[:], 0.0)
                        out_tiles.append(z)
                    return out_tiles
                ch0 = plan_choff[pr]
                djt = sm.tile([P, ktot], f32, tag="dj")
                nc.scalar.dma_start(out=djt[:], in_=dj_d[:, ch0:ch0 + ktot])
                stile = sb.tile([P, ktot, P], f16, tag="S")
                for sj in range(ktot):
                    nc.vector.tensor_scalar(
                        out=stile[:, sj, :], in0=iota_t[:],
                        scalar1=djt[:, sj:sj + 1], scalar2=None,
                        op0=mybir.AluOpType.is_equal)
                itile = sm.tile([P, ktot * 8], i16, tag="it")
                nc.sync.dma_start(out=itile[:], in_=idx16_d[:, ch0 * 8:(ch0 + ktot) * 8])
                g = sb.tile([P, ktot, row_elems], f16, tag="G")
                koff = 0
                for base_id, ks in groups:
                    k = sum(ks)
                    if k == 0:
                        continue
                    src_ap = src_table if base_id == 0 else src_table2
                    nc.gpsimd.dma_gather(
                        out_ap=g[:, koff:koff + k, :],
                        in_ap=src_ap,
                        idxs_ap=itile[:, koff * 8:(koff + k) * 8],
                        num_idxs=k * P, num_idxs_reg=k * P,
                        elem_size=row_elems, single_packet=False,
                        queue_num=gq[0] % 2,
                    )
                    gq[0] += 1
                    koff += k
                # per-block PSUM accumulation over that block's chunks
                for bi, b in enumerate(blocks):
                    agg_ps = psA.tile([P, 512], f32, tag=f"agg_ps{bi}",
                                      name=f"agg_ps{bi}")
                    mm_idx = []
                    koff = 0
                    for base_id, ks in groups:
                        pre = 0
                        for i2, k2 in enumerate(ks):
                            if i2 == bi:
                                mm_idx += list(range(koff + pre, koff + pre + k2))
                            pre += k2
                        koff += sum(ks)
                    if not mm_idx:
                        z = sm.tile([P, width], f16, tag=f"agg{tagsfx}{bi}",
                                    name=f"aggz2{bi}")
                        nc.vector.memset(z[:], 0.0)
                        out_tiles.append(z)
                        continue
                    for n_, j in enumerate(mm_idx):
                        nc.tensor.matmul(
                            out=agg_ps[:, :width],
                            lhsT=stile[:, j, :], rhs=g[:, j, :width],
                            start=(n_ == 0), stop=(n_ == len(mm_idx) - 1),
                        )
                    asb = sm.tile([P, width], f16, tag=f"agg{tagsfx}{bi}",
                                  name=f"asb{bi}")
                    nc.vector.tensor_scalar(
                        out=asb[:], in0=agg_ps[:, :width],
                        scalar1=deginv_t[:, b:b + 1], scalar2=None,
                        op0=mybir.AluOpType.mult,
                    )
                    out_tiles.append(asb)
                return out_tiles

            def layer_123(l, src_rows, src_rows2, fi_chunks, rhs_for_fi, width, row_elems):
                """One SAGE layer with BN+ReLU. rhs_for_fi(q, ns, ne) gives the
                [K, n] rhs AP of the root term for fi-chunk q; aggregation uses
                src_rows tables at `width` features."""
                stats = [sb.tile([P, ntile * 6], f32, tag=f"stats{q}", name=f"stats{q}") for q in range(nfc)]
                for nt in range(ntile):
                    ns, ne = nt * 512, min((nt + 1) * 512, nown)
                    nn = ne - ns
                    # aggregate the (up to) 4 dst blocks of this node tile
                    aggT = (sb.tile([in_f, 512], f16, tag="aggT", name="aggT")
                            if width == in_f else None)
                    aggTq = ([sb.tile([P, 512], f16, tag=f"aggT{q}", name=f"aggT{q}")
                              for q in range(fi_chunks)] if width > in_f else None)
                    pair_tiles = []
                    for pr in (2 * nt, 2 * nt + 1):
                        if pr * 2 < nblk:
                            pair_tiles += aggregate_pair(pr, src_rows, src_rows2,
                                                         width, "sb", row_elems)
                    for bi, b in enumerate(range(nt * 4, min(nt * 4 + 4, nblk))):
                        asb = pair_tiles[bi]
                        tp = psB.tile([P, 512], f16, tag="tp")
                        if width == in_f:
                            nc.tensor.matmul(out=tp[:width, bi * P:(bi + 1) * P],
                                             lhsT=asb[:], rhs=ident[:],
                                             is_transpose=True)
                            nc.vector.tensor_copy(out=aggT[:width, bi * P:(bi + 1) * P],
                                                  in_=tp[:width, bi * P:(bi + 1) * P])
                        else:
                            for q in range(fi_chunks):
                                nc.tensor.matmul(out=tp[:, q * P:(q + 1) * P],
                                                 lhsT=asb[:, q * P:(q + 1) * P],
                                                 rhs=ident[:], is_transpose=True)
                                nc.vector.tensor_copy(out=aggTq[q][:, bi * P:(bi + 1) * P],
                                                      in_=tp[:, q * P:(q + 1) * P])
                    # dense: out^T [fo chunk, nodes]
                    for fo in range(nfc):
                        dps = psC.tile([P, 512], f32, tag="dense")
                        nmm = 2 * fi_chunks
                        mm = 0
                        for q in range(fi_chunks):
                            rhs_agg = (aggT[:width, :nn] if width == in_f
                                       else aggTq[q][:, :nn])
                            nc.tensor.matmul(out=dps[:, :nn],
                                             lhsT=W[("l", l, q)][:, fo * P:(fo + 1) * P],
                                             rhs=rhs_agg, start=(mm == 0),
                                             stop=(mm == nmm - 1))
                            mm += 1
                            nc.tensor.matmul(out=dps[:, :nn],
                                             lhsT=W[("r", l, q)][:, fo * P:(fo + 1) * P],
                                             rhs=rhs_for_fi(q, ns, ne),
                                             start=False, stop=(mm == nmm - 1))
                            mm += 1
                        nc.vector.bn_stats(out=stats[fo][:, nt * 6:(nt + 1) * 6],
                                           in_=dps[:, :nn])
                        nc.vector.tensor_copy(out=preBN[fo][:, ns:ne], in_=dps[:, :nn])
                # ---- BN statistics + cross-core allreduce ----
                pack = sb.tile([P, 8], f32, tag="pack")
                mv = [sb.tile([P, 2], f32, tag=f"mv{q}", name=f"mv{q}") for q in range(nfc)]
                for q in range(nfc):
                    nc.vector.bn_aggr(out=mv[q][:], in_=stats[q][:])
                    # S1 = mean*n_own ; S2 = (var + mean^2)*n_own
                    sq = sb.tile([P, 1], f32, tag="sq")
                    nc.vector.tensor_tensor(out=sq[:], in0=mv[q][:, 0:1],
                                            in1=mv[q][:, 0:1], op=mybir.AluOpType.mult)
                    nc.vector.tensor_tensor(out=sq[:], in0=sq[:], in1=mv[q][:, 1:2],
                                            op=mybir.AluOpType.add)
                    nc.vector.tensor_scalar(out=pack[:, 2 * q:2 * q + 1],
                                            in0=mv[q][:, 0:1], scalar1=float(nown),
                                            scalar2=None, op0=mybir.AluOpType.mult)
                    nc.vector.tensor_scalar(out=pack[:, 2 * q + 1:2 * q + 2],
                                            in0=sq[:], scalar1=float(nown),
                                            scalar2=None, op0=mybir.AluOpType.mult)
                nc.sync.dma_start(out=st_in[l][:, :], in_=pack[:])
                nc.gpsimd.collective_compute(
                    "AllReduce", mybir.AluOpType.add, replica_groups=rg,
                    ins=[st_in[l][:, :]], outs=[st_out[l][:, :]],
                )
                red = sb.tile([P, 8], f32, tag="red")
                nc.sync.dma_start(out=red[:], in_=st_out[l][:, :])
                scale = sb.tile([P, nfc], f32, tag="scale")
                shift = sb.tile([P, nfc], f32, tag="shift")
                inv_n = 1.0 / float(n_nodes)
                for q in range(nfc):
                    mu = sb.tile([P, 1], f32, tag="mu")
                    var = sb.tile([P, 1], f32, tag="var")
                    nc.vector.tensor_scalar(out=mu[:], in0=red[:, 2 * q:2 * q + 1],
                                            scalar1=inv_n, scalar2=None,
                                            op0=mybir.AluOpType.mult)
                    nc.vector.tensor_scalar(out=var[:], in0=red[:, 2 * q + 1:2 * q + 2],
                                            scalar1=inv_n, scalar2=None,
                                            op0=mybir.AluOpType.mult)
                    musq = sb.tile([P, 1], f32, tag="musq")
                    nc.vector.tensor_tensor(out=musq[:], in0=mu[:], in1=mu[:],
                                            op=mybir.AluOpType.mult)
                    nc.vector.tensor_tensor(out=var[:], in0=var[:], in1=musq[:],
                                            op=mybir.AluOpType.subtract)
                    nc.vector.tensor_scalar(out=var[:], in0=var[:], scalar1=EPS,
                                            scalar2=None, op0=mybir.AluOpType.add)
                    nc.vector.reciprocal(out=var[:], in_=var[:])
                    rs = sb.tile([P, 1], f32, tag="rs")
                    nc.scalar.activation(out=rs[:], in_=var[:],
                                         func=mybir.ActivationFunctionType.Sqrt)
                    nc.vector.tensor_tensor(out=scale[:, q:q + 1], in0=rs[:],
                                            in1=gb[("g", l)][:, q:q + 1],
                                            op=mybir.AluOpType.mult)
                    nc.vector.tensor_tensor(out=musq[:], in0=mu[:],
                                            in1=scale[:, q:q + 1],
                                            op=mybir.AluOpType.mult)
                    nc.vector.tensor_tensor(out=shift[:, q:q + 1],
                                            in0=gb[("b", l)][:, q:q + 1], in1=musq[:],
                                            op=mybir.AluOpType.subtract)
                # ---- BN apply + ReLU -> hT (f16), then rows + AllGather ----
                for q in range(nfc):
                    for nt in range(ntile):
                        ns, ne = nt * 512, min((nt + 1) * 512, nown)
                        nc.scalar.activation(
                            out=hT[q][:, ns:ne], in_=preBN[q][:, ns:ne],
                            func=mybir.ActivationFunctionType.Relu,
                            bias=shift[:, q:q + 1], scale=scale[:, q:q + 1],
                        )
                for b in range(nblk):
                    ns, ne = b * P, min((b + 1) * P, nown)
                    tpr = psB.tile([P, 512], f16, tag="tp")
                    for q in range(nfc):
                        nc.tensor.matmul(out=tpr[:, q * P:(q + 1) * P],
                                         lhsT=hT[q][:, b * P:(b + 1) * P],
                                         rhs=ident[:], is_transpose=True)
                    rows = sb.tile([P, hid], f16, tag="rows")
                    nc.vector.tensor_copy(out=rows[:], in_=tpr[:, :hid])
                    nc.sync.dma_start(out=h_own[l][ns:ne, :], in_=rows[:ne - ns, :])
                nc.gpsimd.collective_compute(
                    "AllGather", mybir.AluOpType.bypass, replica_groups=rg,
                    ins=[h_own[l][:, :]], outs=[h_all[l][:, :]],
                )

            # ================= layer 1 =================
            def xT_rhs(q, ns, ne):
                xt = sm.tile([in_f, 512], f16, tag="xTt", name="xTt")
                nc.sync.dma_start(out=xt[:, :ne - ns], in_=xT[:, ns:ne])
                return xt[:, :ne - ns]
            layer_123(1, x16[:, :], x16[BASE2:, :], 1, xT_rhs, in_f, 128)
            # ================= layers 2,3 =================
            for l in (2, 3):
                layer_123(l, h_all[l - 1][:, :], h_all[l - 1][BASE2:, :], nfc,
                          lambda q, ns, ne: hT[q][:, ns:ne], hid, hid)
            # ================= layer 4 =================
            # y = h3 @ Wl4 (transposed), to rows, allgather
            for nt in range(ntile):
                ns, ne = nt * 512, min((nt + 1) * 512, nown)
                nn = ne - ns
                yps = psC.tile([P, 512], f32, tag="dense")
                for q in range(nfc):
                    nc.tensor.matmul(out=yps[:out_f, :nn],
                                     lhsT=W[("l", 4, q)][:, :out_f],
                                     rhs=hT[q][:, ns:ne],
                                     start=(q == 0), stop=(q == nfc - 1))
                ysb = sb.tile([P, 512], f16, tag="ysb")
                nc.vector.tensor_copy(out=ysb[:out_f, :nn], in_=yps[:out_f, :nn])
                for bi in range((nn + P - 1) // P):
                    b0 = bi * P
                    b1 = min(b0 + P, nn)
                    tpy = psB.tile([P, 512], f16, tag="tp")
                    nc.tensor.matmul(out=tpy[:b1 - b0, :out_f],
                                     lhsT=ysb[:out_f, b0:b1],
                                     rhs=ident[:out_f, :out_f],
                                     is_transpose=True)
                    yr = sb.tile([P, 128], f16, tag="yrows")
                    nc.vector.memset(yr[:], 0.0)
                    nc.vector.tensor_copy(out=yr[:b1 - b0, :out_f],
                                          in_=tpy[:b1 - b0, :out_f])
                    nc.sync.dma_start(out=y_own[ns + b0:ns + b1, :],
                                      in_=yr[:b1 - b0, :])
            nc.gpsimd.collective_compute(
                "AllGather", mybir.AluOpType.bypass, replica_groups=rg,
                ins=[y_own[:, :]], outs=[y_all[:, :]],
            )
            # final: out = mean-agg(y) + h3 @ Wr4 + bl4
            for nt in range(ntile):
                ns, ne = nt * 512, min((nt + 1) * 512, nown)
                nn = ne - ns
                agg4T = sb.tile([P, 512], f16, tag="agg4T")
                pair_tiles4 = []
                for pr in (2 * nt, 2 * nt + 1):
                    if pr * 2 < nblk:
                        pair_tiles4 += aggregate_pair(pr, y_all[:, :],
                                                      y_all[BASE2:, :],
                                                      out_f, "4", 128)
                for bi, b in enumerate(range(nt * 4, min(nt * 4 + 4, nblk))):
                    asb = pair_tiles4[bi]
                    tp = psB.tile([P, 512], f16, tag="tp")
                    nc.tensor.matmul(out=tp[:out_f, bi * P:(bi + 1) * P],
                                     lhsT=asb[:], rhs=ident[:], is_transpose=True)
                    nc.vector.tensor_copy(out=agg4T[:out_f, bi * P:(bi + 1) * P],
                                          in_=tp[:out_f, bi * P:(bi + 1) * P])
                ops = psC.tile([P, 512], f32, tag="dense")
                for q in range(nfc):
                    nc.tensor.matmul(out=ops[:out_f, :nn],
                                     lhsT=W[("r", 4, q)][:, :out_f],
                                     rhs=hT[q][:, ns:ne],
                                     start=(q == 0), stop=(q == nfc - 1))
                osb = sb.tile([P, 512], f32, tag="osb")
                nc.vector.tensor_tensor(out=osb[:out_f, :nn], in0=ops[:out_f, :nn],
                                        in1=agg4T[:out_f, :nn],
                                        op=mybir.AluOpType.add)
                nc.vector.tensor_scalar(out=osb[:out_f, :nn], in0=osb[:out_f, :nn],
                                        scalar1=bl4_t[:out_f, 0:1], scalar2=None,
                                        op0=mybir.AluOpType.add)
                for bi in range((nn + P - 1) // P):
                    b0, b1 = bi * P, min(bi * P + P, nn)
                    tpo = psB.tile([P, 512], f32, tag="tp")
                    nc.tensor.matmul(out=tpo[:b1 - b0, :out_f],
                                     lhsT=osb[:out_f, b0:b1],
                                     rhs=ident32[:out_f, :out_f],
                                     is_transpose=True)
                    orow = sb.tile([P, out_f], f32, tag="orow")
                    nc.vector.tensor_copy(out=orow[:b1 - b0, :],
                                          in_=tpo[:b1 - b0, :out_f])
                    nc.sync.dma_start(out=out_d[ns + b0:ns + b1, :],
                                      in_=orow[:b1 - b0, :])
    return nc


# chunk offsets per block, filled by build_inputs (shared plan state)
plan_choff = []


def _prep(plan):
    """Fill global chunk-offset table for the builder."""
    global plan_choff
    plan_choff = []
    off = 0
    for groups in plan.calls:
        plan_choff.append(off)
        off += sum(sum(ks) for _, ks in groups)


def kernel(**inputs):
    x = np.asarray(inputs["x"], np.float32)
    edge_index = np.asarray(inputs["edge_index"])
    n_nodes, in_f = x.shape
    hid = inputs["Wl2"].shape[0]
    out_f = inputs["Wl4"].shape[1]
    nown = n_nodes // NCORES

    src = np.asarray(edge_index[0]).astype(np.int64)
    dst = np.asarray(edge_index[1]).astype(np.int64)
    deg = np.bincount(dst, minlength=n_nodes).astype(np.float32)
    deginv = (1.0 / np.maximum(deg, 1.0)).astype(np.float32)

    plans = _plan_all(n_nodes, edge_index)
    # pad chunk counts to the max across cores so one program fits all
    plans = _pad_plans(plans)
    _prep(plans[0])

    import time as _time
    _t0 = _time.perf_counter()
    nc = build_program(n_nodes, in_f, hid, out_f, plans[0])
    print(f"[kernel] program built in {_time.perf_counter() - _t0:.1f}s", flush=True)
    _t0 = _time.perf_counter()
    nc.compile()
    print(f"[kernel] bacc compile in {_time.perf_counter() - _t0:.1f}s", flush=True)

    x16 = np.zeros((n_nodes, 128), np.float16)
    x16[:, :in_f] = x.astype(np.float16)
    nblk = plans[0].nblk
    pad_n = nblk * P

    in_maps = []
    for c, p in enumerate(plans):
        xTc = np.zeros((in_f, pad_n), np.float16)
        xTc[:, :nown] = x[c * nown:(c + 1) * nown].T.astype(np.float16)
        dg = np.zeros(pad_n, np.float32)
        dg[:nown] = deginv[c * nown:(c + 1) * nown]
        im = {
            "x16": x16, "xT": xTc,
            "idx16": p.idx16 if p.idx16.size else np.zeros((P, 8), np.int16),
            "dj": p.dj16 if p.dj16.size else np.zeros((P, 1), np.float32),
            "deginv": dg,
            "bl4": np.asarray(inputs["bl4"], np.float32),
        }
        for l in (1, 2, 3, 4):
            im[f"Wl{l}"] = np.asarray(inputs[f"Wl{l}"], np.float16)
            im[f"Wr{l}"] = np.asarray(inputs[f"Wr{l}"], np.float16)
        for l in (1, 2, 3):
            im[f"g{l}"] = np.asarray(inputs[f"g{l}"], np.float32)
            im[f"b{l}"] = np.asarray(inputs[f"b{l}"], np.float32)
        in_maps.append(im)

    global LAST_BUILD
    LAST_BUILD = (nc, in_maps)
    res = run_bass_kernel_spmd(nc, in_maps, list(range(NCORES)))
    out = np.concatenate([res.results[c]["out"] for c in range(NCORES)], axis=0)
    return out.astype(np.float32)


def _pad_plans(plans):
    """Pad every core's per-(block,group) chunk count to the cross-core max
    and rebuild idx16/sblk accordingly, so one program serves all cores."""
    npair = plans[0].npair
    kmax = {}
    for pr in range(npair):
        for gi in range(2):
            nb = len(plans[0].calls[pr][gi][1])
            kmax[(pr, gi)] = [max(p.calls[pr][gi][1][i] for p in plans)
                              for i in range(nb)]
    for p in plans:
        idx_vals, dj_vals, calls = [], [], []
        off = 0
        orig_iv = _unwrap_idx(p.idx16, p.totch)
        orig_dj = p.dj16.T.reshape(-1)  # flat [totch*128]
        for pr in range(npair):
            groups = []
            for gi in range(2):
                base_id, ks = p.calls[pr][gi]
                kms = kmax[(pr, gi)]
                for i, (k, km) in enumerate(zip(ks, kms)):
                    iv = np.zeros(km * P, np.int16)
                    dv = np.full(km * P, 200.0, np.float32)
                    if k:
                        iv[:k * P] = orig_iv[off * P:(off + k) * P]
                        dv[:k * P] = orig_dj[off * P:(off + k) * P]
                    off += k
                    idx_vals.append(iv)
                    dj_vals.append(dv)
                groups.append((base_id, list(kms)))
            calls.append(groups)
        p.calls = calls
        p.totch = sum(sum(kmax[(pr, gi)]) for pr in range(npair) for gi in range(2))
        iv = np.concatenate(idx_vals) if idx_vals else np.zeros(0, np.int16)
        w = iv.reshape(-1, 16).T
        p.idx16 = np.tile(w, (8, 1)).copy()
        djf = np.concatenate(dj_vals) if dj_vals else np.zeros(0, np.float32)
        p.dj16 = djf.reshape(-1, P).T.copy()
    return plans


def _unwrap_idx(idx16, totch):
    """Inverse of the 16-partition wrap: [128, totch*8] -> flat [totch*128]."""
    if idx16.size == 0:
        return np.zeros(0, np.int16)
    return idx16[:16, :].T.reshape(-1)

